# revision 14
# baseline (speedup 1.0000x reference)
"""MultiHeadAttention Trainium2 Bass kernel.

Head-sharded tensor parallel across 8 NeuronCores (2 heads/core).
All-transposed dataflow: activations live feature-on-partition so no
on-device activation transposes are needed; the per-head attention
computes S.T = K Q.T directly, softmax is max-free (scores are bounded),
the additive attention bias is applied as a multiply by exp(bias)
(precomputed on host, with key-padding-masked rows zeroed per batch so
no separate mask pass is needed), and the denominator falls out of the
PV matmul via an all-ones 65th lhsT column.

Host side: inputs are pre-transposed / pre-cast to fp16, outputs are
partial sums (row-parallel out projection) summed on host.
"""

import sys

sys.path.insert(0, "/opt/trn_rl_repo")

import numpy as np

B, S, H, NH = 2, 2048, 1024, 16
HD = H // NH            # 64
NCORES = 8
HPC = NH // NCORES      # 2 heads per core
CW = HPC * HD           # 128 = per-core slice width
R = B * S               # 4096 flattened rows
SCALE = float(HD) ** -0.5
F = H // 128            # 8 feature blocks
RC = R // 512           # 8 row chunks
QC = S // 512           # 4 q chunks per batch
KB = S // 128           # 16 k blocks per batch
T = B * KB              # 32 (b, kb) blocks

_CACHE = {}


def _build_module():
    import concourse.bass as bass
    import concourse.tile as tile
    from concourse import bacc, mybir
    from concourse.masks import make_identity

    f16 = mybir.dt.float16
    f32 = mybir.dt.float32
    Exp = mybir.ActivationFunctionType.Exp

    nc = bacc.Bacc(
        "TRN2", target_bir_lowering=False, debug=False, num_devices=NCORES
    )

    # ---- DRAM I/O (per core) ----
    xq = nc.dram_tensor("xq_t", [H, R], f16, kind="ExternalInput").ap()
    xk = nc.dram_tensor("xk_t", [H, R], f16, kind="ExternalInput").ap()
    xv = nc.dram_tensor("xv_t", [H, R], f16, kind="ExternalInput").ap()
    wq = nc.dram_tensor("wq_t", [H, CW], f16, kind="ExternalInput").ap()
    wk = nc.dram_tensor("wk_t", [H, CW], f16, kind="ExternalInput").ap()
    wv = nc.dram_tensor("wv_t", [H, CW], f16, kind="ExternalInput").ap()
    wo = nc.dram_tensor("wo_t", [CW, H], f16, kind="ExternalInput").ap()
    qb = nc.dram_tensor("qb_col", [CW, 1], f32, kind="ExternalInput").ap()
    kb_ = nc.dram_tensor("kb_col", [CW, 1], f32, kind="ExternalInput").ap()
    eb = nc.dram_tensor("eb_t", [B, QC, S, HPC * 512], f16,
                        kind="ExternalInput").ap()
    opart = nc.dram_tensor("o_part", [R, H], f16, kind="ExternalOutput").ap()

    with tile.TileContext(nc) as tc:
        _emit(tc, nc, f16, f32, Exp, make_identity, bass,
              xq, xk, xv, wq, wk, wv, wo, qb, kb_, eb, opart)

    nc.compile()
    return nc


def _emit(tc, nc, f16, f32, Exp, make_identity, bass,
          xq, xk, xv, wq, wk, wv, wo, qb, kb_, eb, opart):
    from contextlib import ExitStack

    with ExitStack() as top:
        consts = top.enter_context(tc.tile_pool(name="consts", bufs=1))
        pers = top.enter_context(tc.tile_pool(name="pers", bufs=1))
        xpool = top.enter_context(tc.tile_pool(name="xin", bufs=3))
        mm = top.enter_context(tc.tile_pool(name="mmpsum", bufs=3,
                                            space="PSUM"))
        cvp_pool = top.enter_context(tc.tile_pool(name="cvpsum", bufs=1,
                                                  space="PSUM"))
        vtp = top.enter_context(tc.tile_pool(name="vt", bufs=2))
        ebp = top.enter_context(tc.tile_pool(name="ebp", bufs=2))
        esp = top.enter_context(tc.tile_pool(name="esp", bufs=3))
        ptp = top.enter_context(tc.tile_pool(name="ptp", bufs=3))
        bcp = top.enter_context(tc.tile_pool(name="bcp", bufs=2))
        rcp = top.enter_context(tc.tile_pool(name="rcp", bufs=1))
        op = top.enter_context(tc.tile_pool(name="op", bufs=2))
        dscr = top.enter_context(tc.tile_pool(name="dscr", bufs=4,
                                              space="DRAM"))

        # ---- tiles for constants / persistent activations ----
        wq_sb = consts.tile([128, F, 128], f16, tag="wq")
        wk_sb = consts.tile([128, F, 128], f16, tag="wk")
        wv_sb = consts.tile([128, F, 128], f16, tag="wv")
        wo_sb = consts.tile([128, H], f16, tag="wo")
        qb_sb = consts.tile([128, 1], f32, tag="qb")
        kb_sb = consts.tile([128, 1], f32, tag="kb")
        ident = consts.tile([128, 128], f16, tag="ident")

        qT_sb = pers.tile([128, R], f16, tag="qT")
        kT_sb = pers.tile([128, R], f16, tag="kT")
        # v in natural layout per (b,kb) block: 64 v cols + all-ones col
        # (row 64 of the PV output = unmasked denominator; masked k rows
        # contribute exactly 0 because eb is zeroed there).
        v_nat = pers.tile([128, T, HPC, 65], f16, tag="vn")
        ctxn = [pers.tile([128, S], f16, tag=f"ctxn{b}", name=f"ctxn{b}")
                for b in range(B)]
        ctx1 = [pers.tile([64, S], f16, tag=f"ctx1{b}", name=f"ctx1{b}")
                for b in range(B)]

        opr = opart.rearrange("(g p) hh -> p g hh", p=128)
        ebr = eb.rearrange("b qc (kb p) m -> p b qc kb m", p=128)
        xqr = xq.rearrange("(f p) r -> p f r", p=128)
        xkr = xk.rearrange("(f p) r -> p f r", p=128)
        xvr = xv.rearrange("(f p) r -> p f r", p=128)
        PIPE = 1            # pending 2-kb groups before PV drain
        op_pend = []

        nc.vector.memset(v_nat, 1.0)
        make_identity(nc, ident)

        # ---------- projection emitters (one rc chunk each) ----------
        def proj_rc(which, rc):
            w_sb, xr, dst, bias_col = {
                "q": (wq_sb, xqr, qT_sb, qb_sb),
                "k": (wk_sb, xkr, kT_sb, kb_sb),
            }[which]
            xt = xpool.tile([128, F, 512], f16, tag="xt",
                            name=f"xt_{which}{rc}")
            nc.sync.dma_start(xt, xr[:, :, rc * 512:(rc + 1) * 512])
            ps = mm.tile([128, 512], f32, tag="sps", name=f"ps_{which}{rc}")
            for f in range(F):
                nc.tensor.matmul(ps, lhsT=w_sb[:, f, :], rhs=xt[:, f, :],
                                 start=(f == 0), stop=(f == F - 1))
            nc.vector.tensor_scalar_add(
                dst[:, rc * 512:(rc + 1) * 512], ps, bias_col)

        def proj_v_rc(rc):
            xt = xpool.tile([128, F, 512], f16, tag="xt", name=f"xt_v{rc}")
            nc.sync.dma_start(xt, xvr[:, :, rc * 512:(rc + 1) * 512])
            ps = mm.tile([128, 512], f32, tag="sps", name=f"ps_v{rc}")
            for f in range(F):
                nc.tensor.matmul(ps, lhsT=wv_sb[:, f, :], rhs=xt[:, f, :],
                                 start=(f == 0), stop=(f == F - 1))
            vt = vtp.tile([128, 512], f16, tag="vt")
            nc.scalar.copy(vt, ps)
            for i in range(4):
                t = rc * 4 + i          # t = b*KB + kb
                tp = mm.tile([128, 128], f16, tag="sps", name=f"tp{rc}_{i}")
                nc.tensor.transpose(tp, vt[:, i * 128:(i + 1) * 128], ident)
                for h in range(HPC):
                    nc.vector.tensor_copy(v_nat[:, t, h, 0:64],
                                          tp[:, h * 64:(h + 1) * 64])

        # ---------- attention chunk emitter ----------
        def attn(qc, b, ebq):
            cvp = cvp_pool.tile([65, HPC, 512], f32, tag="cv",
                                name=f"cv{qc}_{b}")

            def emit_pv(ptt, g):
                for j in range(2):
                    kb = 2 * g + j
                    for h in range(HPC):
                        nc.tensor.matmul(
                            cvp[:, h, :],
                            lhsT=v_nat[:, b * KB + kb, h, :],
                            rhs=ptt[:, j, h, :],
                            start=(kb == 0), stop=(kb == KB - 1))

            pend = []
            est = None
            for kb in range(KB):
                sps = mm.tile([128, HPC, 512], f32, tag="sps",
                              name=f"sps{qc}_{kb}_{b}")
                for h in range(HPC):
                    nc.tensor.matmul(
                        sps[:, h, :],
                        lhsT=kT_sb[h * 64:(h + 1) * 64,
                                   b * S + kb * 128:b * S + (kb + 1) * 128],
                        rhs=qT_sb[h * 64:(h + 1) * 64,
                                  b * S + qc * 512:b * S + (qc + 1) * 512],
                        start=True, stop=True)
                g, half = kb // 2, kb % 2
                if half == 0:
                    est = esp.tile([128, 2, HPC, 512], f16, tag="es",
                                   name=f"es{qc}_{b}_{g}")
                nc.scalar.activation(est[:, half], sps, func=Exp, scale=SCALE)
                if half == 1:
                    ptt = ptp.tile([128, 2, HPC, 512], f16, tag="pt",
                                   name=f"pt{qc}_{b}_{g}")
                    eng = nc.gpsimd if g % 4 == 3 else nc.vector
                    eng.tensor_mul(ptt, est, ebq[:, 2 * g:2 * g + 2, :, :])
                    pend.append((ptt, g))
                    if len(pend) > PIPE:
                        emit_pv(*pend.pop(0))
            for args in pend:
                emit_pv(*args)

            # previous chunk's out-projection (inputs long since ready)
            while len(op_pend) > 2:
                op_pend.pop(0)()

            # 1/denominator (row 64 of cvp), then broadcast to 64 rows
            # via a DRAM round-trip
            rc_sb = rcp.tile([65, HPC, 512], f32, tag="rc")
            nc.vector.reciprocal(rc_sb[64:65, :, :], cvp[64:65, :, :])
            scr = dscr.tile([1, HPC, 512], f32, tag="scr",
                            name=f"scr{qc}_{b}")
            nc.sync.dma_start(scr, rc_sb[64:65, :, :])
            bc = bcp.tile([64, HPC, 512], f32, tag="bc")
            nc.sync.dma_start(bc, scr.to_broadcast((64, HPC, 512)))
            nc.vector.tensor_mul(
                ctxn[b][0:64, qc * 512:(qc + 1) * 512], cvp[0:64, 0, :],
                bc[:, 0, :])
            # h1: lanes 0-63; via ctx1, relocated to partitions 64-127
            nc.vector.tensor_mul(
                ctx1[b][:, qc * 512:(qc + 1) * 512], cvp[0:64, 1, :],
                bc[:, 1, :])
            nc.sync.dma_start(
                ctxn[b][64:128, qc * 512:(qc + 1) * 512],
                ctx1[b][:, qc * 512:(qc + 1) * 512])

            def emit_op(qc=qc, b=b):
                ob_g = op.tile([128, QC, H], f16, tag="ob",
                               name=f"ob{qc}_{b}")
                for ri in range(QC):
                    rb = qc * QC + ri
                    po = mm.tile([128, HPC, 512], f32, tag="sps",
                                 name=f"po{qc}_{b}_{ri}")
                    lhsT = ctxn[b][:, rb * 128:(rb + 1) * 128]
                    nc.tensor.matmul(po[:, 0, :], lhsT=lhsT,
                                     rhs=wo_sb[:, 0:512],
                                     start=True, stop=True)
                    nc.tensor.matmul(po[:, 1, :], lhsT=lhsT,
                                     rhs=wo_sb[:, 512:1024],
                                     start=True, stop=True)
                    nc.scalar.copy(
                        ob_g[:, ri, :].rearrange("p (i j) -> p i j", i=2),
                        po)
                g0 = b * (S // 128) + qc * QC
                nc.sync.dma_start(opr[:, g0:g0 + QC, :], ob_g)
            op_pend.append(emit_op)

        def get_ebq(qc, b):
            ebq = ebp.tile([128, KB, HPC, 512], f16, tag="eb",
                           name=f"ebq{qc}_{b}")
            for g in range(4):
                nc.sync.dma_start(ebq[:, g * 4:(g + 1) * 4, :, :],
                                  ebr[:, b, qc, g * 4:(g + 1) * 4, :])
            return ebq

        # ---------- interleaved schedule ----------
        nc.sync.dma_start(wq_sb, wq.rearrange("(f p) j -> p f j", p=128))
        nc.sync.dma_start(qb_sb, qb)
        proj_rc("q", 0)
        nc.sync.dma_start(wk_sb, wk.rearrange("(f p) j -> p f j", p=128))
        nc.sync.dma_start(kb_sb, kb_)
        for rc in range(4):
            proj_rc("k", rc)
        nc.sync.dma_start(wv_sb, wv.rearrange("(f p) j -> p f j", p=128))
        nc.sync.dma_start(wo_sb, wo)
        for rc in range(4):
            proj_v_rc(rc)
        eb00 = get_ebq(0, 0)

        # first attention chunk overlaps the remaining projections' DMA
        attn(0, 0, eb00)
        proj_rc("q", 4)
        for rc in range(4, 8):
            proj_rc("k", rc)
        for rc in range(4, 8):
            proj_v_rc(rc)
        eb01 = get_ebq(0, 1)
        attn(0, 1, eb01)
        for rc in (1, 2, 3, 5, 6, 7):
            proj_rc("q", rc)

        for qc in range(1, QC):
            for b in range(B):
                ebq = get_ebq(qc, b)
                attn(qc, b, ebq)
        for fn in op_pend:
            fn()


def get_module():
    if "nc" not in _CACHE:
        _CACHE["nc"] = _build_module()
    return _CACHE["nc"]


def make_in_maps(query, key, value, key_padding_mask, bias,
                 q_w, q_b, k_w, k_b, v_w, v_b, o_w, o_b):
    f16 = np.float16
    xq_t = np.ascontiguousarray(query.reshape(R, H).T).astype(f16)
    xk_t = np.ascontiguousarray(key.reshape(R, H).T).astype(f16)
    xv_t = np.ascontiguousarray(value.reshape(R, H).T).astype(f16)

    kpm = np.asarray(key_padding_mask)  # [B, S] bool

    in_maps = []
    for c in range(NCORES):
        hs = slice(c * CW, (c + 1) * CW)
        # eb layout [b, qc, k, i, qi]: exp(bias).T pre-sliced by q chunk,
        # with key-padding-masked k rows zeroed per batch
        ebt = np.empty((B, QC, S, HPC, 512), f16)
        for i in range(HPC):
            h = c * HPC + i
            e = np.exp(np.asarray(bias[0, h], np.float32).T)  # [k, q]
            er = e.reshape(S, QC, 512).transpose(1, 0, 2)     # [qc, k, qi]
            for b in range(B):
                eb_b = er.copy()
                eb_b[:, kpm[b], :] = 0.0
                ebt[b, :, :, i, :] = eb_b.astype(f16)
        ebt = ebt.reshape(B, QC, S, HPC * 512)
        in_maps.append({
            "xq_t": xq_t, "xk_t": xk_t, "xv_t": xv_t,
            "wq_t": np.ascontiguousarray(np.asarray(q_w)[hs].T).astype(f16),
            "wk_t": np.ascontiguousarray(np.asarray(k_w)[hs].T).astype(f16),
            "wv_t": np.ascontiguousarray(np.asarray(v_w)[hs].T).astype(f16),
            "wo_t": np.ascontiguousarray(np.asarray(o_w)[:, hs].T).astype(f16),
            "qb_col": np.asarray(q_b, np.float32)[hs].reshape(CW, 1).copy(),
            "kb_col": np.asarray(k_b, np.float32)[hs].reshape(CW, 1).copy(),
            "eb_t": ebt,
        })
    return in_maps


def assemble_output(results, v_b, o_w, o_b):
    acc = np.zeros((R, H), np.float32)
    for res in results:
        acc += np.asarray(res["o_part"], np.float32)
    corr = np.asarray(v_b, np.float32) @ np.asarray(o_w, np.float32).T \
        + np.asarray(o_b, np.float32)
    acc += corr[None, :]
    return acc.reshape(B, S, H).astype(np.float32)


def kernel(**inputs):
    from concourse.bass_utils import run_bass_kernel_spmd

    nc = get_module()
    in_maps = make_in_maps(**inputs)
    res = run_bass_kernel_spmd(nc, in_maps, list(range(NCORES)))
    return assemble_output(res.results, inputs["v_b"], inputs["o_w"],
                           inputs["o_b"])


# revision 15
# speedup vs baseline: 1.3220x; 1.3220x over previous
"""MultiHeadAttention Trainium2 Bass kernel.

Head-sharded tensor parallel across 8 NeuronCores (2 heads/core).
All-transposed dataflow: activations live feature-on-partition so no
on-device activation transposes are needed; the per-head attention
computes S.T = K Q.T directly, softmax is max-free (scores are bounded),
the additive attention bias is applied as a multiply by exp(bias)
(precomputed on host, with key-padding-masked rows zeroed per batch so
no separate mask pass is needed), and the denominator falls out of the
PV matmul via an all-ones 65th lhsT column.

Host side: inputs are pre-transposed / pre-cast to fp16, outputs are
partial sums (row-parallel out projection) summed on host.
"""

import sys

sys.path.insert(0, "/opt/trn_rl_repo")

import numpy as np

B, S, H, NH = 2, 2048, 1024, 16
HD = H // NH            # 64
NCORES = 8
HPC = NH // NCORES      # 2 heads per core
CW = HPC * HD           # 128 = per-core slice width
R = B * S               # 4096 flattened rows
SCALE = float(HD) ** -0.5
F = H // 128            # 8 feature blocks
RC = R // 512           # 8 row chunks
QC = S // 512           # 4 q chunks per batch
KB = S // 128           # 16 k blocks per batch
T = B * KB              # 32 (b, kb) blocks

_CACHE = {}


def _build_module():
    import concourse.bass as bass
    import concourse.tile as tile
    from concourse import bacc, mybir
    from concourse.masks import make_identity

    f16 = mybir.dt.float16
    f32 = mybir.dt.float32
    Exp = mybir.ActivationFunctionType.Exp

    nc = bacc.Bacc(
        "TRN2", target_bir_lowering=False, debug=False, num_devices=NCORES
    )

    # ---- DRAM I/O (per core) ----
    xq = nc.dram_tensor("xq_t", [H, R], f16, kind="ExternalInput").ap()
    xk = nc.dram_tensor("xk_t", [H, R], f16, kind="ExternalInput").ap()
    xv = nc.dram_tensor("xv_t", [H, R], f16, kind="ExternalInput").ap()
    wq = nc.dram_tensor("wq_t", [H, CW], f16, kind="ExternalInput").ap()
    wk = nc.dram_tensor("wk_t", [H, CW], f16, kind="ExternalInput").ap()
    wv = nc.dram_tensor("wv_t", [H, CW], f16, kind="ExternalInput").ap()
    wo = nc.dram_tensor("wo_t", [CW, H], f16, kind="ExternalInput").ap()
    qb = nc.dram_tensor("qb_col", [CW, 1], f32, kind="ExternalInput").ap()
    kb_ = nc.dram_tensor("kb_col", [CW, 1], f32, kind="ExternalInput").ap()
    eb = nc.dram_tensor("eb_t", [B, QC, S, HPC * 512], f16,
                        kind="ExternalInput").ap()
    opart = nc.dram_tensor("o_part", [R, H], f16, kind="ExternalOutput").ap()

    with tile.TileContext(nc) as tc:
        _emit(tc, nc, f16, f32, Exp, make_identity, bass,
              xq, xk, xv, wq, wk, wv, wo, qb, kb_, eb, opart)

    nc.compile()
    return nc


def _emit(tc, nc, f16, f32, Exp, make_identity, bass,
          xq, xk, xv, wq, wk, wv, wo, qb, kb_, eb, opart):
    from contextlib import ExitStack

    with ExitStack() as top:
        consts = top.enter_context(tc.tile_pool(name="consts", bufs=1))
        pers = top.enter_context(tc.tile_pool(name="pers", bufs=1))
        xpool = top.enter_context(tc.tile_pool(name="xin", bufs=2))
        mm = top.enter_context(tc.tile_pool(name="mmpsum", bufs=3,
                                            space="PSUM"))
        cvp_pool = top.enter_context(tc.tile_pool(name="cvpsum", bufs=1,
                                                  space="PSUM"))
        vtp = top.enter_context(tc.tile_pool(name="vt", bufs=2))
        ebp = top.enter_context(tc.tile_pool(name="ebp", bufs=2))
        esp = top.enter_context(tc.tile_pool(name="esp", bufs=3))
        ptp = top.enter_context(tc.tile_pool(name="ptp", bufs=3))
        bcp = top.enter_context(tc.tile_pool(name="bcp", bufs=2))
        rcp = top.enter_context(tc.tile_pool(name="rcp", bufs=1))
        op = top.enter_context(tc.tile_pool(name="op", bufs=2))
        dscr = top.enter_context(tc.tile_pool(name="dscr", bufs=4,
                                              space="DRAM"))

        # ---- tiles for constants / persistent activations ----
        wq_sb = consts.tile([128, F, 128], f16, tag="wq")
        wk_sb = consts.tile([128, F, 128], f16, tag="wk")
        wv_sb = consts.tile([128, F, 128], f16, tag="wv")
        wo_sb = consts.tile([128, H], f16, tag="wo")
        qb_sb = consts.tile([128, 1], f32, tag="qb")
        kb_sb = consts.tile([128, 1], f32, tag="kb")
        ident = consts.tile([128, 128], f16, tag="ident")

        # Per-head q tiles, zero-padded on the other head's partitions so
        # the scores matmul contracts over all 128 partitions (64-partition
        # matmuls stream rhs at half rate on TRN2).
        qT_h = [pers.tile([128, R], f16, tag=f"qT{h}", name=f"qT{h}")
                for h in range(HPC)]
        kT_sb = pers.tile([128, R], f16, tag="kT")
        # v in natural layout per (b,kb) block: 64 v cols + all-ones col
        # (row 64 of the PV output = unmasked denominator; masked k rows
        # contribute exactly 0 because eb is zeroed there).
        v_nat = pers.tile([128, T, HPC, 65], f16, tag="vn")
        ctxn = [pers.tile([128, S], f16, tag=f"ctxn{b}", name=f"ctxn{b}")
                for b in range(B)]
        ctx1 = [pers.tile([64, S], f16, tag=f"ctx1{b}", name=f"ctx1{b}")
                for b in range(B)]

        opr = opart.rearrange("(g p) hh -> p g hh", p=128)
        ebr = eb.rearrange("b qc (kb p) m -> p b qc kb m", p=128)
        xqr = xq.rearrange("(f p) r -> p f r", p=128)
        xkr = xk.rearrange("(f p) r -> p f r", p=128)
        xvr = xv.rearrange("(f p) r -> p f r", p=128)
        PIPE = 1            # pending 2-kb groups before PV drain
        op_pend = []

        nc.vector.memset(v_nat, 1.0)
        nc.vector.memset(qT_h[0][64:128, :], 0.0)
        nc.vector.memset(qT_h[1][0:64, :], 0.0)
        make_identity(nc, ident)

        # ---------- projection emitters (one rc chunk each) ----------
        def proj_rc(which, rc):
            w_sb, xr, bias_col = {
                "q": (wq_sb, xqr, qb_sb),
                "k": (wk_sb, xkr, kb_sb),
            }[which]
            xt = xpool.tile([128, F, 512], f16, tag="xt",
                            name=f"xt_{which}{rc}")
            nc.sync.dma_start(xt, xr[:, :, rc * 512:(rc + 1) * 512])
            ps = mm.tile([128, 512], f32, tag="sps", name=f"ps_{which}{rc}")
            for f in range(F):
                nc.tensor.matmul(ps, lhsT=w_sb[:, f, :], rhs=xt[:, f, :],
                                 start=(f == 0), stop=(f == F - 1))
            cols = slice(rc * 512, (rc + 1) * 512)
            if which == "k":
                nc.vector.tensor_scalar_add(kT_sb[:, cols], ps, bias_col)
            else:
                for h in range(HPC):
                    rows = slice(h * 64, (h + 1) * 64)
                    nc.vector.tensor_scalar_add(
                        qT_h[h][rows, cols], ps[rows, :], bias_col[rows, :])

        def proj_v_rc(rc):
            xt = xpool.tile([128, F, 512], f16, tag="xt", name=f"xt_v{rc}")
            nc.sync.dma_start(xt, xvr[:, :, rc * 512:(rc + 1) * 512])
            ps = mm.tile([128, 512], f32, tag="sps", name=f"ps_v{rc}")
            for f in range(F):
                nc.tensor.matmul(ps, lhsT=wv_sb[:, f, :], rhs=xt[:, f, :],
                                 start=(f == 0), stop=(f == F - 1))
            vt = vtp.tile([128, 512], f16, tag="vt")
            nc.scalar.copy(vt, ps)
            for i in range(4):
                t = rc * 4 + i          # t = b*KB + kb
                tp = mm.tile([128, 128], f16, tag="sps", name=f"tp{rc}_{i}")
                nc.tensor.transpose(tp, vt[:, i * 128:(i + 1) * 128], ident)
                for h in range(HPC):
                    nc.vector.tensor_copy(v_nat[:, t, h, 0:64],
                                          tp[:, h * 64:(h + 1) * 64])

        # ---------- attention chunk emitter ----------
        def attn(qc, b, ebq):
            cvp = cvp_pool.tile([65, HPC, 512], f32, tag="cv",
                                name=f"cv{qc}_{b}")

            def emit_pv(ptt, g):
                for j in range(2):
                    kb = 2 * g + j
                    for h in range(HPC):
                        nc.tensor.matmul(
                            cvp[:, h, :],
                            lhsT=v_nat[:, b * KB + kb, h, :],
                            rhs=ptt[:, j, h, :],
                            start=(kb == 0), stop=(kb == KB - 1))

            pend = []
            est = None
            for kb in range(KB):
                sps = mm.tile([128, HPC, 512], f32, tag="sps",
                              name=f"sps{qc}_{kb}_{b}")
                for h in range(HPC):
                    nc.tensor.matmul(
                        sps[:, h, :],
                        lhsT=kT_sb[:, b * S + kb * 128:b * S + (kb + 1) * 128],
                        rhs=qT_h[h][:, b * S + qc * 512:b * S + (qc + 1) * 512],
                        start=True, stop=True)
                g, half = kb // 2, kb % 2
                if half == 0:
                    est = esp.tile([128, 2, HPC, 512], f16, tag="es",
                                   name=f"es{qc}_{b}_{g}")
                nc.scalar.activation(est[:, half], sps, func=Exp, scale=SCALE)
                if half == 1:
                    ptt = ptp.tile([128, 2, HPC, 512], f16, tag="pt",
                                   name=f"pt{qc}_{b}_{g}")
                    eng = nc.gpsimd if g % 4 == 3 else nc.vector
                    eng.tensor_mul(ptt, est, ebq[:, 2 * g:2 * g + 2, :, :])
                    pend.append((ptt, g))
                    if len(pend) > PIPE:
                        emit_pv(*pend.pop(0))
            for args in pend:
                emit_pv(*args)

            # previous chunk's out-projection (inputs long since ready)
            while len(op_pend) > 2:
                op_pend.pop(0)()

            # 1/denominator (row 64 of cvp), then broadcast to 64 rows
            # via a DRAM round-trip
            rc_sb = rcp.tile([65, 2, HPC, 512], f32, tag="rc")
            Ln = Exp.__class__.Ln
            nc.scalar.activation(rc_sb[64:65, 0], cvp[64:65, :, :], func=Ln)
            nc.scalar.activation(rc_sb[64:65, 1], rc_sb[64:65, 0],
                                 func=Exp, scale=-1.0)
            scr = dscr.tile([1, HPC, 512], f32, tag="scr",
                            name=f"scr{qc}_{b}")
            nc.sync.dma_start(scr, rc_sb[64:65, 1])
            bc = bcp.tile([64, HPC, 512], f32, tag="bc")
            nc.sync.dma_start(bc, scr.to_broadcast((64, HPC, 512)))
            nc.vector.tensor_mul(
                ctxn[b][0:64, qc * 512:(qc + 1) * 512], cvp[0:64, 0, :],
                bc[:, 0, :])
            # h1: lanes 0-63; via ctx1, relocated to partitions 64-127
            nc.vector.tensor_mul(
                ctx1[b][:, qc * 512:(qc + 1) * 512], cvp[0:64, 1, :],
                bc[:, 1, :])
            nc.sync.dma_start(
                ctxn[b][64:128, qc * 512:(qc + 1) * 512],
                ctx1[b][:, qc * 512:(qc + 1) * 512])

            def emit_op(qc=qc, b=b):
                ob_g = op.tile([128, QC, H], f16, tag="ob",
                               name=f"ob{qc}_{b}")
                for ri in range(QC):
                    rb = qc * QC + ri
                    po = mm.tile([128, HPC, 512], f32, tag="sps",
                                 name=f"po{qc}_{b}_{ri}")
                    lhsT = ctxn[b][:, rb * 128:(rb + 1) * 128]
                    nc.tensor.matmul(po[:, 0, :], lhsT=lhsT,
                                     rhs=wo_sb[:, 0:512],
                                     start=True, stop=True)
                    nc.tensor.matmul(po[:, 1, :], lhsT=lhsT,
                                     rhs=wo_sb[:, 512:1024],
                                     start=True, stop=True)
                    evac = nc.scalar.copy if ri % 2 == 0 \
                        else nc.vector.tensor_copy
                    evac(ob_g[:, ri, :].rearrange("p (i j) -> p i j", i=2),
                         po)
                g0 = b * (S // 128) + qc * QC
                nc.sync.dma_start(opr[:, g0:g0 + QC, :], ob_g)
            op_pend.append(emit_op)

        def get_ebq(qc, b):
            ebq = ebp.tile([128, KB, HPC, 512], f16, tag="eb",
                           name=f"ebq{qc}_{b}")
            for g in range(4):
                nc.sync.dma_start(ebq[:, g * 4:(g + 1) * 4, :, :],
                                  ebr[:, b, qc, g * 4:(g + 1) * 4, :])
            return ebq

        # ---------- interleaved schedule ----------
        nc.sync.dma_start(wq_sb, wq.rearrange("(f p) j -> p f j", p=128))
        nc.sync.dma_start(qb_sb, qb)
        proj_rc("q", 0)
        nc.sync.dma_start(wk_sb, wk.rearrange("(f p) j -> p f j", p=128))
        nc.sync.dma_start(kb_sb, kb_)
        for rc in range(4):
            proj_rc("k", rc)
        nc.sync.dma_start(wv_sb, wv.rearrange("(f p) j -> p f j", p=128))
        nc.sync.dma_start(wo_sb, wo)
        for rc in range(4):
            proj_v_rc(rc)
        eb00 = get_ebq(0, 0)

        # first attention chunk overlaps the remaining projections' DMA
        attn(0, 0, eb00)
        proj_rc("q", 4)
        for rc in range(4, 8):
            proj_rc("k", rc)
        for rc in range(4, 8):
            proj_v_rc(rc)
        eb01 = get_ebq(0, 1)
        attn(0, 1, eb01)
        for rc in (1, 2, 3, 5, 6, 7):
            proj_rc("q", rc)

        for qc in range(1, QC):
            for b in range(B):
                ebq = get_ebq(qc, b)
                attn(qc, b, ebq)
        for fn in op_pend:
            fn()


def get_module():
    if "nc" not in _CACHE:
        _CACHE["nc"] = _build_module()
    return _CACHE["nc"]


def make_in_maps(query, key, value, key_padding_mask, bias,
                 q_w, q_b, k_w, k_b, v_w, v_b, o_w, o_b):
    f16 = np.float16
    xq_t = np.ascontiguousarray(query.reshape(R, H).T).astype(f16)
    xk_t = np.ascontiguousarray(key.reshape(R, H).T).astype(f16)
    xv_t = np.ascontiguousarray(value.reshape(R, H).T).astype(f16)

    kpm = np.asarray(key_padding_mask)  # [B, S] bool

    in_maps = []
    for c in range(NCORES):
        hs = slice(c * CW, (c + 1) * CW)
        # eb layout [b, qc, k, i, qi]: exp(bias).T pre-sliced by q chunk,
        # with key-padding-masked k rows zeroed per batch
        ebt = np.empty((B, QC, S, HPC, 512), f16)
        for i in range(HPC):
            h = c * HPC + i
            e = np.exp(np.asarray(bias[0, h], np.float32).T)  # [k, q]
            er = e.reshape(S, QC, 512).transpose(1, 0, 2)     # [qc, k, qi]
            for b in range(B):
                eb_b = er.copy()
                eb_b[:, kpm[b], :] = 0.0
                ebt[b, :, :, i, :] = eb_b.astype(f16)
        ebt = ebt.reshape(B, QC, S, HPC * 512)
        in_maps.append({
            "xq_t": xq_t, "xk_t": xk_t, "xv_t": xv_t,
            "wq_t": np.ascontiguousarray(np.asarray(q_w)[hs].T).astype(f16),
            "wk_t": np.ascontiguousarray(np.asarray(k_w)[hs].T).astype(f16),
            "wv_t": np.ascontiguousarray(np.asarray(v_w)[hs].T).astype(f16),
            "wo_t": np.ascontiguousarray(np.asarray(o_w)[:, hs].T).astype(f16),
            "qb_col": np.asarray(q_b, np.float32)[hs].reshape(CW, 1).copy(),
            "kb_col": np.asarray(k_b, np.float32)[hs].reshape(CW, 1).copy(),
            "eb_t": ebt,
        })
    return in_maps


def assemble_output(results, v_b, o_w, o_b):
    acc = np.zeros((R, H), np.float32)
    for res in results:
        acc += np.asarray(res["o_part"], np.float32)
    corr = np.asarray(v_b, np.float32) @ np.asarray(o_w, np.float32).T \
        + np.asarray(o_b, np.float32)
    acc += corr[None, :]
    return acc.reshape(B, S, H).astype(np.float32)


def kernel(**inputs):
    from concourse.bass_utils import run_bass_kernel_spmd

    nc = get_module()
    in_maps = make_in_maps(**inputs)
    res = run_bass_kernel_spmd(nc, in_maps, list(range(NCORES)))
    return assemble_output(res.results, inputs["v_b"], inputs["o_w"],
                           inputs["o_b"])
